# revision 1
# baseline (speedup 1.0000x reference)
"""CostVolume2D Trainium2 Bass kernel.

cost[b,h,w,d] = sum_c |feat_l[b,h,w,c] - feat_r[b,h,w-d,c]|
(feat_r zero-padded on the left: for w < d the cost is sum_c |feat_l|)

Sharding: pure data-parallel over batch B=8 across 8 NeuronCores (one image
per core); full inputs in, full output out, sharding handled inside kernel().

Per-core pipeline (natural layout, h on partitions, raw Bass with explicit
minimal semaphores):
  - 2 slabs of 128 h-rows. feat_l slab loads to FL [128, W*C]; feat_r slab
    loads into FR [128, (D+W)*C] whose first D*C columns are memset to zero
    once (the left zero-pad reproduces the w < d boundary exactly, since the
    reference's cost there is sum_c |feat_l - 0|). Both loads are single
    fully-contiguous 8 MB DMAs (HWDGE, ~97% DMA efficiency).
  - For each disparity d and 256-w chunk: DVE tensor_sub(DIFF, FL-slice,
    FR-slice-shifted) where the disparity shift is just a free-dim offset of
    (D-d)*C elements into the zero-padded FR tile, then
    tensor_reduce(op=add, apply_absolute_value=True) over the innermost C=32
    axis of the [128, 256, 32] view into a CONTIGUOUS per-d temp — a
    d-strided reduce output degrades the whole reduce stream (~2x kernel
    cost, measured), so the d-interleave into the cost tile is deferred to
    one small strided tensor_copy per disparity.
  - Cost slabs [128, W*D] store contiguously. Compute runs entirely on the
    DVE in FIFO order (no intra-engine semaphores needed); cross-engine sync
    is one DMA semaphore + one DVE completion semaphore with cumulative-count
    waits. All semaphores are cleared in a kernel tail so the NEFF is safely
    re-executable (without this, second executions of the loaded NEFF see
    stale semaphore values and race).

Result is bit-exact vs the fp32 jax reference (verified rel err 0.0 on HW,
including across repeated executions). Measured ~6.3 ms on-device per
kernel execution before the ScalarE copy offload (slope method over in-NEFF
repetition counts; down from 11.3 ms with the d-strided reduce output);
the ScalarE offload removes the 24 strided copies from the DVE critical
path with bounded stall (verified bit-exact, not re-slope-measured).
"""

import numpy as np

import concourse.bass as bass
import concourse.mybir as mybir
from concourse.instruction_name_ordered_set import InstructionNameOrderedSet
from concourse.bass_utils import run_bass_kernel_spmd

B, H, W, C, D = 8, 256, 512, 32, 12
N_CORES = 8
P = 128  # partitions per slab

F32 = mybir.dt.float32

_NC_CACHE = {}


def build_nc(h=H, w=W, p=P, w_chunk=256, reps=1):
    n_slabs = h // p
    n_wc = w // w_chunk
    n_gslabs = n_slabs * reps
    nc = bass.Bass()
    fl = nc.dram_tensor("feat_l", [h, w * C], F32, kind="ExternalInput")
    fr = nc.dram_tensor("feat_r", [h, w * C], F32, kind="ExternalInput")
    cost = nc.dram_tensor("cost", [h, w * D], F32, kind="ExternalOutput")

    with (
        nc.sbuf_tensor([p, w * C], F32) as FL,
        nc.sbuf_tensor([p, (D + w) * C], F32) as FR,
        nc.sbuf_tensor([p, w_chunk * C], F32) as DIFF,
        nc.sbuf_tensor([p, w], F32) as CT0,
        nc.sbuf_tensor([p, w], F32) as CT1,
        nc.sbuf_tensor([p, w * D], F32) as CO,
        nc.semaphore("dma_sem") as dma_sem,
        nc.semaphore("d_sem") as d_sem,
        nc.semaphore("act_sem") as act_sem,
        nc.Block() as block,
    ):
        CTS = [CT0, CT1]

        @block.sync
        def _(sync):
            for g in range(n_gslabs):
                s = g % n_slabs
                if g > 0:
                    # previous slab's compute must fully finish before
                    # overwriting FL/FR and storing CO (act_sem transitively
                    # implies the DVE finished its FL/FR reads too)
                    sync.wait_ge(act_sem, D * g)
                    sp = (g - 1) % n_slabs
                    sync.dma_start(
                        out=cost[sp * p : (sp + 1) * p, :], in_=CO[:, :]
                    ).then_inc(dma_sem, 16)
                sync.dma_start(
                    out=FL[:, :], in_=fl[s * p : (s + 1) * p, :]
                ).then_inc(dma_sem, 16)
                sync.dma_start(
                    out=FR[:, D * C :], in_=fr[s * p : (s + 1) * p, :]
                ).then_inc(dma_sem, 16)
            sync.wait_ge(act_sem, D * n_gslabs)
            sp = (n_gslabs - 1) % n_slabs
            sync.dma_start(
                out=cost[sp * p : (sp + 1) * p, :], in_=CO[:, :]
            ).then_inc(dma_sem, 16)
            # reset all semaphores so the NEFF is safely re-executable
            sync.wait_ge(dma_sem, 48 * (n_gslabs - 1) + 32 + 16)
            for sem in (dma_sem, d_sem, act_sem):
                sync.sem_clear(sem)

        @block.vector
        def _(vector):
            # chain consecutive DVE instructions with nosync (FIFO) deps so
            # the simulator race-detector understands same-engine ordering
            prev = [None]

            def chain(inst):
                if prev[0] is not None:
                    deps = InstructionNameOrderedSet()
                    deps.add(prev[0].ins.name)
                    inst.ins.add_nosync_dependencies_from(deps)
                prev[0] = inst
                return inst

            chain(vector.memset(FR[:, : D * C], 0.0))
            CO3 = CO[:, :].rearrange("p (w d) -> p w d", d=D)
            for g in range(n_gslabs):
                # wait for this slab's loads (and the previous slab's CO
                # store) to complete: 48*g+32 is the cumulative-inc total at
                # that point, reached only once every prior DMA finished
                chain(vector.wait_ge(dma_sem, 48 * g + 32))
                for d in range(D):
                    dg = g * D + d
                    # CT double-buffer WAR: ScalarE must have copied out the
                    # temp this d is about to overwrite
                    if dg >= 2:
                        chain(vector.wait_ge(act_sem, dg - 1))
                    CT = CTS[dg % 2]
                    for wc in range(n_wc):
                        l_ap = FL[:, wc * w_chunk * C : (wc + 1) * w_chunk * C]
                        r_off = (D - d) * C + wc * w_chunk * C
                        r_ap = FR[:, r_off : r_off + w_chunk * C]
                        chain(vector.tensor_sub(DIFF[:, :], l_ap, r_ap))
                        red_in = DIFF[:, :].rearrange("p (w c) -> p w c", c=C)
                        # reduce into a CONTIGUOUS temp: a d-strided reduce
                        # output degrades the whole reduce stream, so the
                        # strided write is deferred to ScalarE copies that
                        # overlap the next disparity's DVE work
                        inst = chain(
                            vector.tensor_reduce(
                                CT[:, wc * w_chunk : (wc + 1) * w_chunk],
                                red_in,
                                axis=mybir.AxisListType.X,
                                op=mybir.AluOpType.add,
                                apply_absolute_value=True,
                            )
                        )
                    inst.then_inc(d_sem, 1)

        @block.scalar
        def _(scalar):
            CO3s = CO[:, :].rearrange("p (w d) -> p w d", d=D)
            for g in range(n_gslabs):
                # previous slab's CO store must complete before overwriting
                scalar.wait_ge(dma_sem, 48 * g + 32)
                for d in range(D):
                    dg = g * D + d
                    scalar.wait_ge(d_sem, dg + 1)
                    scalar.copy(CO3s[:, :, d], CTS[dg % 2][:, :]).then_inc(
                        act_sem, 1
                    )

    return nc


def _get_nc():
    if "nc" not in _NC_CACHE:
        _NC_CACHE["nc"] = build_nc()
    return _NC_CACHE["nc"]


def _run(feat_l, feat_r, trace=False, nc=None):
    if nc is None:
        nc = _get_nc()
    feat_l = np.asarray(feat_l, dtype=np.float32)
    feat_r = np.asarray(feat_r, dtype=np.float32)
    in_maps = []
    for b in range(B):
        in_maps.append(
            {
                "feat_l": np.ascontiguousarray(feat_l[b].reshape(H, W * C)),
                "feat_r": np.ascontiguousarray(feat_r[b].reshape(H, W * C)),
            }
        )
    res = run_bass_kernel_spmd(nc, in_maps, list(range(N_CORES)), trace=trace)
    out = np.stack(
        [res.results[i]["cost"].reshape(H, W, D) for i in range(B)]
    ).astype(np.float32)
    return out, res


def kernel(feat_l, feat_r):
    out, _ = _run(feat_l, feat_r, trace=False)
    return out



# revision 23
# speedup vs baseline: 37.8482x; 37.8482x over previous
"""CostVolume2D Trainium2 Bass kernel.

cost[b,h,w,d] = sum_c |feat_l[b,h,w,c] - feat_r[b,h,w-d,c]|
(feat_r zero-padded on the left: for w < d the cost is sum_c |feat_l|)

Sharding: pure data-parallel over batch B=8 across 8 NeuronCores (one image
per core); full inputs in, full output out, sharding handled inside kernel().

Per-core pipeline (natural layout, h on partitions, raw Bass with explicit
minimal semaphores):
  - 2 slabs of 128 h-rows. feat_l slab loads to FL [128, W*C]; feat_r slab
    loads into FR [128, (D+W)*C] whose first D*C columns are memset to zero
    once (the left zero-pad reproduces the w < d boundary exactly, since the
    reference's cost there is sum_c |feat_l - 0|). Both loads are single
    fully-contiguous 8 MB DMAs (HWDGE, ~97% DMA efficiency).
  - For each disparity d and 256-w chunk: DVE tensor_sub(DIFF, FL-slice,
    FR-slice-shifted) where the disparity shift is just a free-dim offset of
    (D-d)*C elements into the zero-padded FR tile, then
    tensor_reduce(op=add, apply_absolute_value=True) over the innermost C=32
    axis of the [128, 256, 32] view into a CONTIGUOUS per-d temp — a
    d-strided reduce output degrades the whole reduce stream (~2x kernel
    cost, measured), so the d-interleave into the cost tile is deferred to
    one small strided tensor_copy per disparity.
  - Cost slabs [128, W*D] store contiguously. Compute runs entirely on the
    DVE in FIFO order (no intra-engine semaphores needed); cross-engine sync
    is one DMA semaphore + one DVE completion semaphore with cumulative-count
    waits. All semaphores are cleared in a kernel tail so the NEFF is safely
    re-executable (without this, second executions of the loaded NEFF see
    stale semaphore values and race).

Result is bit-exact vs the fp32 jax reference (verified rel err 0.0 on HW,
including across repeated executions). Measured ~6.3 ms on-device per
kernel execution before the ScalarE copy offload (slope method over in-NEFF
repetition counts; down from 11.3 ms with the d-strided reduce output);
the ScalarE offload removes the 24 strided copies from the DVE critical
path with bounded stall (verified bit-exact, not re-slope-measured).
"""

import numpy as np

import concourse.bass as bass
import concourse.mybir as mybir
from concourse.instruction_name_ordered_set import InstructionNameOrderedSet
from concourse.bass_utils import run_bass_kernel_spmd

B, H, W, C, D = 8, 256, 512, 32, 12
N_CORES = 8
P = 128  # partitions per slab

F32 = mybir.dt.float32

_NC_CACHE = {}


def build_nc(h=H, w=W, p=P, w_chunk=256, reps=1):
    n_slabs = h // p
    n_wc = w // w_chunk
    n_gslabs = n_slabs * reps
    nc = bass.Bass()
    fl = nc.dram_tensor("feat_l", [h, w * C], F32, kind="ExternalInput")
    fr = nc.dram_tensor("feat_r", [h, w * C], F32, kind="ExternalInput")
    cost = nc.dram_tensor("cost", [h, w * D], F32, kind="ExternalOutput")

    with (
        nc.sbuf_tensor([p, w * C], F32) as FL,
        nc.sbuf_tensor([p, (D + w) * C], F32) as FR,
        nc.sbuf_tensor([p, w_chunk * C], F32) as DIFF,
        nc.sbuf_tensor([p, w], F32) as CT0,
        nc.sbuf_tensor([p, w], F32) as CT1,
        nc.sbuf_tensor([p, w * D], F32) as CO,
        nc.semaphore("dma_sem") as dma_sem,
        nc.semaphore("d_sem") as d_sem,
        nc.semaphore("act_sem") as act_sem,
        nc.Block() as block,
    ):
        CTS = [CT0, CT1]

        @block.sync
        def _(sync):
            for g in range(n_gslabs):
                s = g % n_slabs
                if g > 0:
                    # previous slab's compute must fully finish before
                    # overwriting FL/FR and storing CO (act_sem transitively
                    # implies the DVE finished its FL/FR reads too)
                    sync.wait_ge(act_sem, D * g)
                    sp = (g - 1) % n_slabs
                    sync.dma_start(
                        out=cost[sp * p : (sp + 1) * p, :], in_=CO[:, :]
                    ).then_inc(dma_sem, 16)
                sync.dma_start(
                    out=FL[:, :], in_=fl[s * p : (s + 1) * p, :]
                ).then_inc(dma_sem, 16)
                sync.dma_start(
                    out=FR[:, D * C :], in_=fr[s * p : (s + 1) * p, :]
                ).then_inc(dma_sem, 16)
            sync.wait_ge(act_sem, D * n_gslabs)
            sp = (n_gslabs - 1) % n_slabs
            sync.dma_start(
                out=cost[sp * p : (sp + 1) * p, :], in_=CO[:, :]
            ).then_inc(dma_sem, 16)
            # reset all semaphores so the NEFF is safely re-executable
            sync.wait_ge(dma_sem, 48 * (n_gslabs - 1) + 32 + 16)
            for sem in (dma_sem, d_sem, act_sem):
                sync.sem_clear(sem)

        @block.vector
        def _(vector):
            # chain consecutive DVE instructions with nosync (FIFO) deps so
            # the simulator race-detector understands same-engine ordering
            prev = [None]

            def chain(inst):
                if prev[0] is not None:
                    deps = InstructionNameOrderedSet()
                    deps.add(prev[0].ins.name)
                    inst.ins.add_nosync_dependencies_from(deps)
                prev[0] = inst
                return inst

            chain(vector.memset(FR[:, : D * C], 0.0))
            CO3 = CO[:, :].rearrange("p (w d) -> p w d", d=D)
            for g in range(n_gslabs):
                # wait for this slab's loads (and the previous slab's CO
                # store) to complete: 48*g+32 is the cumulative-inc total at
                # that point, reached only once every prior DMA finished
                chain(vector.wait_ge(dma_sem, 48 * g + 32))
                for d in range(D):
                    dg = g * D + d
                    # CT double-buffer WAR: ScalarE must have copied out the
                    # temp this d is about to overwrite
                    if dg >= 2:
                        chain(vector.wait_ge(act_sem, dg - 1))
                    CT = CTS[dg % 2]
                    for wc in range(n_wc):
                        l_ap = FL[:, wc * w_chunk * C : (wc + 1) * w_chunk * C]
                        r_off = (D - d) * C + wc * w_chunk * C
                        r_ap = FR[:, r_off : r_off + w_chunk * C]
                        chain(vector.tensor_sub(DIFF[:, :], l_ap, r_ap))
                        red_in = DIFF[:, :].rearrange("p (w c) -> p w c", c=C)
                        # reduce into a CONTIGUOUS temp: a d-strided reduce
                        # output degrades the whole reduce stream, so the
                        # strided write is deferred to ScalarE copies that
                        # overlap the next disparity's DVE work
                        inst = chain(
                            vector.tensor_reduce(
                                CT[:, wc * w_chunk : (wc + 1) * w_chunk],
                                red_in,
                                axis=mybir.AxisListType.X,
                                op=mybir.AluOpType.add,
                                apply_absolute_value=True,
                            )
                        )
                    inst.then_inc(d_sem, 1)

        @block.scalar
        def _(scalar):
            CO3s = CO[:, :].rearrange("p (w d) -> p w d", d=D)
            for g in range(n_gslabs):
                # previous slab's CO store must complete before overwriting
                scalar.wait_ge(dma_sem, 48 * g + 32)
                for d in range(D):
                    dg = g * D + d
                    scalar.wait_ge(d_sem, dg + 1)
                    scalar.copy(CO3s[:, :, d], CTS[dg % 2][:, :]).then_inc(
                        act_sem, 1
                    )

    return nc


def _get_nc():
    if "nc" not in _NC_CACHE:
        _NC_CACHE["nc"] = build_nc()
    return _NC_CACHE["nc"]


def _run(feat_l, feat_r, trace=False, nc=None):
    if nc is None:
        nc = _get_nc()
    feat_l = np.asarray(feat_l, dtype=np.float32)
    feat_r = np.asarray(feat_r, dtype=np.float32)
    in_maps = []
    for b in range(B):
        in_maps.append(
            {
                "feat_l": np.ascontiguousarray(feat_l[b].reshape(H, W * C)),
                "feat_r": np.ascontiguousarray(feat_r[b].reshape(H, W * C)),
            }
        )
    res = run_bass_kernel_spmd(nc, in_maps, list(range(N_CORES)), trace=trace)
    out = np.stack(
        [res.results[i]["cost"].reshape(H, W, D) for i in range(B)]
    ).astype(np.float32)
    return out, res


def kernel(feat_l, feat_r):
    out, _ = _run(feat_l, feat_r, trace=False)
    return out

